# revision 13
# baseline (speedup 1.0000x reference)
"""Trainium2 Bass kernel for nn_MultiHeadAttention_87239375716860.

MHA with the reference's quirk: softmax normalizes over the HEADS axis
(score[q,k,b,h], softmax(axis=-1) -> over h), not over keys.

Sharding (no collectives): 8 cores = 4 batches x 2 query-halves.
Core d = 2*b + qc handles batch b, queries [qc*1024, (qc+1)*1024).
Each core projects its batch's full K/V (duplicated between the two
q-half cores) so the softmax-over-heads and the k-contraction are
fully local.

Per-core dataflow (layouts picked so NO on-device transposes are
needed; the host supplies x^T and W^T):
  qT[e,s] = Ws^T-proj of xq^T (scale 1/8 + bias folded in), bf16
  kT[e,s] = Ws^T-proj of xk^T, bf16
  v[s,e]  = proj of xv^T (bias added), bf16
  per (q-block, k-tile, head): s^T[k,q] = kT-chunk @ qT-chunk (psum)
  e = exp(s^T) -> bf16;  Z[k,q] = sum_h e;  a = e * (1/Z)  (bf16)
  o^T[e,q] += v-chunk^T @ a_h   (psum accumulate over k-tiles)
  y[q,f]  = o^T-slices @ Wo^T + b_o -> HBM   (fp32r matmuls)

HW hazard note: matmuls whose SBUF operands or PSUM outputs sit at
partition offset 64 crash the PE when the offset alternates between
consecutive matmuls.  All matmuls here therefore use full-height
(offset-0) operands; per-head selection is done by zero-padding the
unused head-half of q (scores) and v (attn@v) - zeros contribute
nothing to the contraction / accumulation.
"""

import numpy as np

SEQ = 2048
BATCH = 4
D = 1024
H = 16
DK = 64
QCH = 1024          # queries per core
NCORES = 8
QB = 256            # q-block size in phase 2
NQB = QCH // QB     # 4
NKT = SEQ // 128    # 16 k-tiles

_CACHE = {}


def _build_bass():
    """Build + schedule the per-core Bass program (SPMD: same NEFF on all
    8 cores, different input data)."""
    from contextlib import ExitStack

    import concourse.tile as tile
    from concourse import bacc, mybir

    f32 = mybir.dt.float32
    f32r = mybir.dt.float32r
    bf16 = mybir.dt.bfloat16
    AF = mybir.ActivationFunctionType

    nc = bacc.Bacc("TRN2", target_bir_lowering=False, debug=False,
                   num_devices=NCORES)

    xqT_d = nc.dram_tensor("xqT", [D, QCH], f32r, kind="ExternalInput").ap()
    xkT_d = nc.dram_tensor("xkT", [D, SEQ], f32r, kind="ExternalInput").ap()
    xvT_d = nc.dram_tensor("xvT", [D, SEQ], f32r, kind="ExternalInput").ap()
    wsT_d = nc.dram_tensor("wsT", [D, D], f32r, kind="ExternalInput").ap()
    woT_d = nc.dram_tensor("woT", [D, D], f32r, kind="ExternalInput").ap()
    bq_d = nc.dram_tensor("bq", [128, 8], f32, kind="ExternalInput").ap()
    bk_d = nc.dram_tensor("bk", [128, 8], f32, kind="ExternalInput").ap()
    bvb_d = nc.dram_tensor("bvb", [128, D], f32, kind="ExternalInput").ap()
    bob_d = nc.dram_tensor("bob", [128, D], f32, kind="ExternalInput").ap()
    out_d = nc.dram_tensor("out", [QCH, D], f32, kind="ExternalOutput").ap()

    with tile.TileContext(nc) as tc, ExitStack() as ctx:
        # ---------------- pools ----------------
        persist = ctx.enter_context(tc.tile_pool(name="persist", bufs=1))
        psum_s = ctx.enter_context(tc.tile_pool(name="psum_s", space="PSUM", bufs=2))
        psum_o = ctx.enter_context(tc.tile_pool(name="psum_o", space="PSUM", bufs=1))

        # persistent SBUF tensors.  `wio` holds Ws^T during phase 1 and is
        # overwritten with Wo^T once the projections no longer need Ws.
        wio = persist.tile([128, 8, D], f32r, name="wio")
        # qT stored twice with the other head-half zeroed (see module
        # doc), interleaved [p, chunk, parity, s] so a head-pair's scores
        # are one N=512 matmul.
        qT2 = persist.tile([128, 8, 2, QCH], bf16, name="qT2")
        kT = persist.tile([128, 8, SEQ], bf16, name="kT")
        # v per (kt, chunk) stored as [E | Z | O]: even head's lhsT is
        # cols 0:2 ([real | zeros]), odd head's is cols 1:3 ([zeros |
        # real]) -- both contiguous, sharing one zero block.
        vz = persist.tile([128, NKT, 8, 3, DK], bf16, name="vz")
        bq_t = persist.tile([128, 8], f32, name="bq_t")
        bk_t = persist.tile([128, 8], f32, name="bk_t")
        bvb_t = persist.tile([128, 8, 2, DK], f32, name="bvb_t")

        nc.sync.dma_start(wio[:], wsT_d.rearrange("(c p) e -> p c e", p=128))
        nc.gpsimd.memset(qT2[64:128, :, 0, :], 0.0)
        nc.gpsimd.memset(qT2[0:64, :, 1, :], 0.0)
        nc.gpsimd.memset(vz[:, :, :, 1, :], 0.0)
        nc.sync.dma_start(bq_t[:], bq_d)
        nc.sync.dma_start(bk_t[:], bk_d)
        nc.sync.dma_start(bvb_t[:], bvb_d.rearrange("p (c t k) -> p c t k",
                                                    c=8, t=2))

        # ---------------- phase 1: projections ----------------
        with tc.tile_pool(name="stream", bufs=1) as stream:
            # qT[e,s]: lhsT = wsT (d-part, e-cols), rhs = xqT (d-part, s-cols)
            for sb in range(QCH // 512):
                xq_s = stream.tile([128, 8, 512], f32r, tag="xs", bufs=2,
                                   name=f"xq{sb}")
                nc.sync.dma_start(
                    xq_s[:], xqT_d[:, sb * 512:(sb + 1) * 512]
                    .rearrange("(c p) s -> p c s", p=128))
                for c in range(8):
                    ps = psum_s.tile([128, 1024], f32, tag="s",
                                     name=f"psq{sb}_{c}")
                    for dch in range(8):
                        nc.tensor.matmul(
                            ps[:, 0:512],
                            wio[:, dch, c * 128:(c + 1) * 128],
                            xq_s[:, dch, :],
                            start=(dch == 0), stop=(dch == 7))
                    nc.scalar.activation(
                        qT2[0:64, c, 0, sb * 512:(sb + 1) * 512],
                        ps[0:64, 0:512],
                        AF.Identity, bias=bq_t[0:64, c:c + 1], scale=0.125)
                    nc.scalar.activation(
                        qT2[64:128, c, 1, sb * 512:(sb + 1) * 512],
                        ps[64:128, 0:512],
                        AF.Identity, bias=bq_t[64:128, c:c + 1], scale=0.125)

            # kT[e,s]
            for sb in range(SEQ // 512):
                xk_s = stream.tile([128, 8, 512], f32r, tag="xs", bufs=2,
                                   name=f"xk{sb}")
                nc.sync.dma_start(
                    xk_s[:], xkT_d[:, sb * 512:(sb + 1) * 512]
                    .rearrange("(c p) s -> p c s", p=128))
                for c in range(8):
                    ps = psum_s.tile([128, 1024], f32, tag="s",
                                     name=f"psk{sb}_{c}")
                    for dch in range(8):
                        nc.tensor.matmul(
                            ps[:, 0:512],
                            wio[:, dch, c * 128:(c + 1) * 128],
                            xk_s[:, dch, :],
                            start=(dch == 0), stop=(dch == 7))
                    nc.scalar.activation(
                        kT[:, c, sb * 512:(sb + 1) * 512], ps[:, 0:512],
                        AF.Identity, bias=bk_t[:, c:c + 1])

            # v[s,e] bf16 (x2, complementary head-halves zeroed)
            for kch in range(NKT):
                xv_s = stream.tile([128, 8, 128], f32r, tag="xv", bufs=2,
                                   name=f"xv{kch}")
                nc.sync.dma_start(
                    xv_s[:], xvT_d[:, kch * 128:(kch + 1) * 128]
                    .rearrange("(c p) k -> p c k", p=128))
                ps = psum_s.tile([128, 1024], f32, tag="s", name=f"psv{kch}")
                for eb in range(2):
                    for dch in range(8):
                        nc.tensor.matmul(
                            ps[:, eb * 512:(eb + 1) * 512],
                            xv_s[:, dch, :],
                            wio[:, dch, eb * 512:(eb + 1) * 512],
                            start=(dch == 0), stop=(dch == 7))
                pv = ps.rearrange("p (c t k) -> p c t k", c=8, t=2)
                nc.vector.tensor_add(vz[:, kch, :, 0, :], pv[:, :, 0, :],
                                     bvb_t[:, :, 0, :])
                nc.vector.tensor_add(vz[:, kch, :, 2, :], pv[:, :, 1, :],
                                     bvb_t[:, :, 1, :])

        # Ws no longer needed -- load Wo^T into the same tile.
        nc.sync.dma_start(wio[:], woT_d.rearrange("(c p) e -> p c e", p=128))

        # ---------------- phase 2+3: attention + out-proj ----------------
        with tc.tile_pool(name="work", bufs=1) as work:
            bob_t = work.tile([128, D], f32, tag="bob", bufs=1, name="bob_t")
            nc.sync.dma_start(bob_t[:], bob_d)
            for qb in range(NQB):
                q0 = qb * QB
                oT_ps = psum_o.tile([128, 8 * QB], f32, tag="ot",
                                    name=f"ot{qb}")
                for kt in range(NKT):
                    e = work.tile([128, H, QB], bf16, tag="e", bufs=3,
                                  name=f"e{qb}_{kt}")
                    # scores (4 heads = 2 chunk-pairs per psum tile) + exp.
                    # One N=512 matmul per chunk covers its even+odd head
                    # (qT2 parity axis); the unused head-half of q is zero.
                    for hg in range(4):
                        ps = psum_s.tile([128, 1024], f32, tag="s",
                                         name=f"pss{qb}_{kt}_{hg}")
                        for cl in range(2):
                            c = hg * 2 + cl
                            nc.tensor.matmul(
                                ps[:, cl * 512:(cl + 1) * 512],
                                kT[:, c, kt * 128:(kt + 1) * 128],
                                qT2[:, c, :, q0:q0 + QB],
                                start=True, stop=True)
                        nc.scalar.activation(
                            e[:, hg * 4:(hg + 1) * 4, :], ps[:, :], AF.Exp)
                    # Z = sum over heads (tree); R = 1/Z
                    t1 = work.tile([128, 2, 4, QB], bf16, tag="t1", bufs=2,
                                   name=f"t1_{qb}_{kt}")
                    nc.vector.tensor_add(t1[:, 0, :, :], e[:, 0:4, :],
                                         e[:, 4:8, :])
                    nc.vector.tensor_add(t1[:, 1, :, :], e[:, 8:12, :],
                                         e[:, 12:16, :])
                    t2 = work.tile([128, 4, QB], bf16, tag="t2", bufs=1,
                                   name=f"t2_{qb}_{kt}")
                    nc.gpsimd.tensor_add(t2[:], t1[:, 0, :, :], t1[:, 1, :, :])
                    t3 = work.tile([128, 2, QB], bf16, tag="t3", bufs=1,
                                   name=f"t3_{qb}_{kt}")
                    nc.gpsimd.tensor_add(t3[:], t2[:, 0:2, :], t2[:, 2:4, :])
                    zf = work.tile([128, QB], f32, tag="zf", bufs=2,
                                   name=f"zf{qb}_{kt}")
                    nc.vector.tensor_add(zf[:], t3[:, 0, :], t3[:, 1, :])
                    rf = work.tile([128, QB], f32, tag="rf", bufs=2,
                                   name=f"rf{qb}_{kt}")
                    nc.vector.reciprocal_approx_fast(rf[:], zf[:])
                    rb = work.tile([128, QB], bf16, tag="rb", bufs=1,
                                   name=f"rb{qb}_{kt}")
                    nc.vector.tensor_copy(rb[:], rf[:])
                    # a = e * R  (in place), R broadcast over heads;
                    # split so attn@v on the first half starts earlier
                    nc.vector.tensor_mul(
                        e[:, 0:8, :], e[:, 0:8, :],
                        rb[:].unsqueeze(1).broadcast_to([128, 8, QB]))
                    nc.vector.tensor_mul(
                        e[:, 8:16, :], e[:, 8:16, :],
                        rb[:].unsqueeze(1).broadcast_to([128, 8, QB]))
                    # o^T[e,q] accumulation, full height; the unused
                    # head-half of v is zero so it adds nothing.
                    # PSUM bank = h//4; start/stop on first/last writer.
                    for h in range(H):
                        c, t0 = h // 2, h % 2
                        nc.tensor.matmul(
                            oT_ps[:, c * QB:(c + 1) * QB],
                            vz[:, kt, c, t0:t0 + 2, :],
                            e[:, h, :],
                            start=(kt == 0 and h % 4 == 0),
                            stop=(kt == NKT - 1 and h % 4 == 3),
                            skip_group_check=True)
                # phase 3: evacuate o^T, out-projection, bias, store
                for qs in range(QB // 128):
                    ot_qs = work.tile([128, 8, 128], f32r, tag="ot_sb",
                                      bufs=1, name=f"otsb{qb}_{qs}")
                    for c in range(8):
                        nc.scalar.activation(
                            ot_qs[:, c, :],
                            oT_ps[:, c * QB + qs * 128:
                                  c * QB + (qs + 1) * 128],
                            AF.Copy)
                    y_ps = psum_s.tile([128, 1024], f32, tag="s",
                                       name=f"psy{qb}_{qs}")
                    for fb in range(2):
                        for c in range(8):
                            nc.tensor.matmul(
                                y_ps[:, fb * 512:(fb + 1) * 512],
                                ot_qs[:, c, :],
                                wio[:, c, fb * 512:(fb + 1) * 512],
                                start=(c == 0), stop=(c == 7))
                    y_sb = work.tile([128, 1024], f32, tag="y", bufs=2,
                                     name=f"y{qb}_{qs}")
                    nc.vector.tensor_add(y_sb[:], y_ps[:], bob_t[:])
                    nc.sync.dma_start(
                        out_d[q0 + qs * 128: q0 + (qs + 1) * 128, :], y_sb[:])

    nc.compile()
    return nc


def _get_nc():
    if "nc" not in _CACHE:
        _CACHE["nc"] = _build_bass()
    return _CACHE["nc"]


def _make_in_maps(query, key, value, W_split, b_split, W_o, b_o):
    query = np.asarray(query, np.float32)
    key = np.asarray(key, np.float32)
    value = np.asarray(value, np.float32)
    W_split = np.asarray(W_split, np.float32)
    b_split = np.asarray(b_split, np.float32)
    W_o = np.asarray(W_o, np.float32)
    b_o = np.asarray(b_o, np.float32)

    wsT = np.ascontiguousarray(W_split.T)
    woT = np.ascontiguousarray(W_o.T)
    bq = np.ascontiguousarray((b_split / 8.0).reshape(8, 128).T)
    bk = np.ascontiguousarray(b_split.reshape(8, 128).T)
    bvb = np.ascontiguousarray(np.broadcast_to(b_split, (128, D)))
    bob = np.ascontiguousarray(np.broadcast_to(b_o, (128, D)))

    in_maps = []
    for d in range(NCORES):
        b, qc = d // 2, d % 2
        xqT = np.ascontiguousarray(query[qc * QCH:(qc + 1) * QCH, b, :].T)
        xkT = np.ascontiguousarray(key[:, b, :].T)
        xvT = np.ascontiguousarray(value[:, b, :].T)
        in_maps.append({
            "xqT": xqT, "xkT": xkT, "xvT": xvT,
            "wsT": wsT, "woT": woT,
            "bq": bq, "bk": bk, "bvb": bvb, "bob": bob,
        })
    return in_maps


def kernel_with_results(trace=False, **inputs):
    from concourse.bass_utils import run_bass_kernel_spmd

    nc = _get_nc()
    in_maps = _make_in_maps(**inputs)
    res = run_bass_kernel_spmd(nc, in_maps, core_ids=list(range(NCORES)),
                               trace=trace)
    out = np.empty((SEQ, BATCH, D), np.float32)
    for d in range(NCORES):
        b, qc = d // 2, d % 2
        out[qc * QCH:(qc + 1) * QCH, b, :] = res.results[d]["out"]
    return out, res


def kernel(**inputs):
    out, _ = kernel_with_results(trace=False, **inputs)
    return out


# revision 14
# speedup vs baseline: 1.9046x; 1.9046x over previous
"""Trainium2 Bass kernel for nn_MultiHeadAttention_87239375716860.

MHA with the reference's quirk: softmax normalizes over the HEADS axis
(score[q,k,b,h], softmax(axis=-1) -> over h), not over keys.

Sharding (no collectives): 8 cores = 4 batches x 2 query-halves.
Core d = 2*b + qc handles batch b, queries [qc*1024, (qc+1)*1024).
Each core projects its batch's full K/V (duplicated between the two
q-half cores) so the softmax-over-heads and the k-contraction are
fully local.

Per-core dataflow (layouts picked so NO on-device transposes are
needed; the host supplies x^T and W^T):
  qT[e,s] = Ws^T-proj of xq^T (scale 1/8 + bias folded in), bf16
  kT[e,s] = Ws^T-proj of xk^T, bf16
  v[s,e]  = proj of xv^T (bias added), bf16
  per (q-block, k-tile, head): s^T[k,q] = kT-chunk @ qT-chunk (psum)
  e = exp(s^T) -> bf16;  Z[k,q] = sum_h e;  a = e * (1/Z)  (bf16)
  o^T[e,q] += v-chunk^T @ a_h   (psum accumulate over k-tiles)
  y[q,f]  = o^T-slices @ Wo^T + b_o -> HBM   (fp32r matmuls)

HW hazard note: matmuls whose SBUF operands or PSUM outputs sit at
partition offset 64 crash the PE when the offset alternates between
consecutive matmuls.  All matmuls here therefore use full-height
(offset-0) operands; per-head selection is done by zero-padding the
unused head-half of q (scores) and v (attn@v) - zeros contribute
nothing to the contraction / accumulation.
"""

import numpy as np

SEQ = 2048
BATCH = 4
D = 1024
H = 16
DK = 64
QCH = 1024          # queries per core
NCORES = 8
QB = 256            # q-block size in phase 2
NQB = QCH // QB     # 4
NKT = SEQ // 128    # 16 k-tiles

_CACHE = {}


def _build_bass():
    """Build + schedule the per-core Bass program (SPMD: same NEFF on all
    8 cores, different input data)."""
    from contextlib import ExitStack

    import concourse.tile as tile
    from concourse import bacc, mybir

    f32 = mybir.dt.float32
    f32r = mybir.dt.float32r
    bf16 = mybir.dt.bfloat16
    AF = mybir.ActivationFunctionType

    nc = bacc.Bacc("TRN2", target_bir_lowering=False, debug=False,
                   num_devices=NCORES)

    xqT_d = nc.dram_tensor("xqT", [D, QCH], f32r, kind="ExternalInput").ap()
    xkT_d = nc.dram_tensor("xkT", [D, SEQ], f32r, kind="ExternalInput").ap()
    xvT_d = nc.dram_tensor("xvT", [D, SEQ], f32r, kind="ExternalInput").ap()
    wsT_d = nc.dram_tensor("wsT", [D, D], f32r, kind="ExternalInput").ap()
    woT_d = nc.dram_tensor("woT", [D, D], f32r, kind="ExternalInput").ap()
    bq_d = nc.dram_tensor("bq", [128, 8], f32, kind="ExternalInput").ap()
    bk_d = nc.dram_tensor("bk", [128, 8], f32, kind="ExternalInput").ap()
    bvb_d = nc.dram_tensor("bvb", [128, D], f32, kind="ExternalInput").ap()
    bob_d = nc.dram_tensor("bob", [128, D], f32, kind="ExternalInput").ap()
    out_d = nc.dram_tensor("out", [QCH, D], f32, kind="ExternalOutput").ap()

    with tile.TileContext(nc) as tc, ExitStack() as ctx:
        # ---------------- pools ----------------
        persist = ctx.enter_context(tc.tile_pool(name="persist", bufs=1))
        psum_s = ctx.enter_context(tc.tile_pool(name="psum_s", space="PSUM", bufs=2))
        psum_o = ctx.enter_context(tc.tile_pool(name="psum_o", space="PSUM", bufs=1))

        # persistent SBUF tensors.  `wio` holds Ws^T during phase 1 and is
        # overwritten with Wo^T once the projections no longer need Ws.
        wio = persist.tile([128, 8, D], f32r, name="wio")
        # qT stored twice with the other head-half zeroed (see module
        # doc), interleaved [p, chunk, parity, s] so a head-pair's scores
        # are one N=512 matmul.
        qT2 = persist.tile([128, 8, 2, QCH], bf16, name="qT2")
        kT = persist.tile([128, 8, SEQ], bf16, name="kT")
        # v per (kt, chunk) stored as [E | Z | O]: even head's lhsT is
        # cols 0:2 ([real | zeros]), odd head's is cols 1:3 ([zeros |
        # real]) -- both contiguous, sharing one zero block.
        vz = persist.tile([128, NKT, 8, 3, DK], bf16, name="vz")
        bq_t = persist.tile([128, 8], f32, name="bq_t")
        bk_t = persist.tile([128, 8], f32, name="bk_t")
        bvb_t = persist.tile([128, 8, 2, DK], f32, name="bvb_t")

        nc.sync.dma_start(wio[:], wsT_d.rearrange("(c p) e -> p c e", p=128))
        nc.gpsimd.memset(qT2[64:128, :, 0, :], 0.0)
        nc.gpsimd.memset(qT2[0:64, :, 1, :], 0.0)
        nc.gpsimd.memset(vz[:, :, :, 1, :], 0.0)
        nc.sync.dma_start(bq_t[:], bq_d)
        nc.sync.dma_start(bk_t[:], bk_d)
        nc.sync.dma_start(bvb_t[:], bvb_d.rearrange("p (c t k) -> p c t k",
                                                    c=8, t=2))

        # ---------------- phase 1: projections ----------------
        with tc.tile_pool(name="stream", bufs=1) as stream:
            # qT[e,s]: lhsT = wsT (d-part, e-cols), rhs = xqT (d-part, s-cols)
            for sb in range(QCH // 512):
                xq_s = stream.tile([128, 8, 512], f32r, tag="xs", bufs=2,
                                   name=f"xq{sb}")
                nc.sync.dma_start(
                    xq_s[:], xqT_d[:, sb * 512:(sb + 1) * 512]
                    .rearrange("(c p) s -> p c s", p=128))
                for c in range(8):
                    ps = psum_s.tile([128, 1024], f32, tag="s",
                                     name=f"psq{sb}_{c}")
                    for dch in range(8):
                        nc.tensor.matmul(
                            ps[:, 0:512],
                            wio[:, dch, c * 128:(c + 1) * 128],
                            xq_s[:, dch, :],
                            start=(dch == 0), stop=(dch == 7))
                    nc.scalar.activation(
                        qT2[0:64, c, 0, sb * 512:(sb + 1) * 512],
                        ps[0:64, 0:512],
                        AF.Identity, bias=bq_t[0:64, c:c + 1], scale=0.125)
                    nc.scalar.activation(
                        qT2[64:128, c, 1, sb * 512:(sb + 1) * 512],
                        ps[64:128, 0:512],
                        AF.Identity, bias=bq_t[64:128, c:c + 1], scale=0.125)

            # kT[e,s]
            for sb in range(SEQ // 512):
                xk_s = stream.tile([128, 8, 512], f32r, tag="xs", bufs=2,
                                   name=f"xk{sb}")
                nc.sync.dma_start(
                    xk_s[:], xkT_d[:, sb * 512:(sb + 1) * 512]
                    .rearrange("(c p) s -> p c s", p=128))
                for c in range(8):
                    ps = psum_s.tile([128, 1024], f32, tag="s",
                                     name=f"psk{sb}_{c}")
                    for dch in range(8):
                        nc.tensor.matmul(
                            ps[:, 0:512],
                            wio[:, dch, c * 128:(c + 1) * 128],
                            xk_s[:, dch, :],
                            start=(dch == 0), stop=(dch == 7))
                    nc.scalar.activation(
                        kT[:, c, sb * 512:(sb + 1) * 512], ps[:, 0:512],
                        AF.Identity, bias=bk_t[:, c:c + 1])

            # v[s,e] bf16 (x2, complementary head-halves zeroed)
            for kch in range(NKT):
                xv_s = stream.tile([128, 8, 128], f32r, tag="xv", bufs=2,
                                   name=f"xv{kch}")
                nc.sync.dma_start(
                    xv_s[:], xvT_d[:, kch * 128:(kch + 1) * 128]
                    .rearrange("(c p) k -> p c k", p=128))
                ps = psum_s.tile([128, 1024], f32, tag="s", name=f"psv{kch}")
                for eb in range(2):
                    for dch in range(8):
                        nc.tensor.matmul(
                            ps[:, eb * 512:(eb + 1) * 512],
                            xv_s[:, dch, :],
                            wio[:, dch, eb * 512:(eb + 1) * 512],
                            start=(dch == 0), stop=(dch == 7))
                pv = ps.rearrange("p (c t k) -> p c t k", c=8, t=2)
                nc.vector.tensor_add(vz[:, kch, :, 0, :], pv[:, :, 0, :],
                                     bvb_t[:, :, 0, :])
                nc.vector.tensor_add(vz[:, kch, :, 2, :], pv[:, :, 1, :],
                                     bvb_t[:, :, 1, :])

        # Ws no longer needed -- load Wo^T into the same tile.
        nc.sync.dma_start(wio[:], woT_d.rearrange("(c p) e -> p c e", p=128))

        # ---------------- phase 2+3: attention + out-proj ----------------
        with tc.tile_pool(name="work", bufs=1) as work:
            bob_t = work.tile([128, D], f32, tag="bob", bufs=1, name="bob_t")
            nc.sync.dma_start(bob_t[:], bob_d)
            for qb in range(NQB):
                q0 = qb * QB
                oT_ps = psum_o.tile([128, 8 * QB], f32, tag="ot",
                                    name=f"ot{qb}")
                for kt in range(NKT):
                    e = work.tile([128, H, QB], bf16, tag="e", bufs=3,
                                  name=f"e{qb}_{kt}")
                    # scores (4 heads = 2 chunk-pairs per psum tile) + exp.
                    # One N=512 matmul per chunk covers its even+odd head
                    # (qT2 parity axis); the unused head-half of q is zero.
                    for hg in range(4):
                        ps = psum_s.tile([128, 1024], f32, tag="s",
                                         name=f"pss{qb}_{kt}_{hg}")
                        for cl in range(2):
                            c = hg * 2 + cl
                            nc.tensor.matmul(
                                ps[:, cl * 512:(cl + 1) * 512],
                                kT[:, c, kt * 128:(kt + 1) * 128],
                                qT2[:, c, :, q0:q0 + QB],
                                start=True, stop=True)
                        nc.scalar.activation(
                            e[:, hg * 4:(hg + 1) * 4, :], ps[:, :], AF.Exp)
                    # Z = sum over heads (tree); R = 1/Z
                    t1 = work.tile([128, 2, 4, QB], bf16, tag="t1", bufs=2,
                                   name=f"t1_{qb}_{kt}")
                    nc.vector.tensor_add(t1[:, 0, :, :], e[:, 0:4, :],
                                         e[:, 4:8, :])
                    nc.vector.tensor_add(t1[:, 1, :, :], e[:, 8:12, :],
                                         e[:, 12:16, :])
                    t2 = work.tile([128, 4, QB], bf16, tag="t2", bufs=1,
                                   name=f"t2_{qb}_{kt}")
                    nc.gpsimd.tensor_add(t2[:], t1[:, 0, :, :], t1[:, 1, :, :])
                    t3 = work.tile([128, 2, QB], bf16, tag="t3", bufs=1,
                                   name=f"t3_{qb}_{kt}")
                    nc.gpsimd.tensor_add(t3[:], t2[:, 0:2, :], t2[:, 2:4, :])
                    zf = work.tile([128, QB], f32, tag="zf", bufs=2,
                                   name=f"zf{qb}_{kt}")
                    nc.vector.tensor_add(zf[:], t3[:, 0, :], t3[:, 1, :])
                    rf = work.tile([128, QB], f32, tag="rf", bufs=2,
                                   name=f"rf{qb}_{kt}")
                    nc.vector.reciprocal_approx_fast(rf[:], zf[:])
                    rb = work.tile([128, QB], bf16, tag="rb", bufs=1,
                                   name=f"rb{qb}_{kt}")
                    nc.vector.tensor_copy(rb[:], rf[:])
                    # a = e * R  (in place), R broadcast over heads;
                    # split so attn@v on the first half starts earlier
                    nc.vector.tensor_mul(
                        e[:, 0:8, :], e[:, 0:8, :],
                        rb[:].unsqueeze(1).broadcast_to([128, 8, QB]))
                    nc.vector.tensor_mul(
                        e[:, 8:16, :], e[:, 8:16, :],
                        rb[:].unsqueeze(1).broadcast_to([128, 8, QB]))
                    # o^T[e,q] accumulation, full height; the unused
                    # head-half of v is zero so it adds nothing.
                    # PSUM bank = h//4; start/stop on first/last writer.
                    for h in range(H):
                        c, t0 = h // 2, h % 2
                        nc.tensor.matmul(
                            oT_ps[:, c * QB:(c + 1) * QB],
                            vz[:, kt, c, t0:t0 + 2, :],
                            e[:, h, :],
                            start=(kt == 0 and h % 4 == 0),
                            stop=(kt == NKT - 1 and h % 4 == 3),
                            skip_group_check=True)
                # phase 3: evacuate o^T, out-projection, bias, store
                for qs in range(QB // 128):
                    ot_qs = work.tile([128, 8, 128], f32r, tag="ot_sb",
                                      bufs=1, name=f"otsb{qb}_{qs}")
                    for c in range(8):
                        nc.scalar.activation(
                            ot_qs[:, c, :],
                            oT_ps[:, c * QB + qs * 128:
                                  c * QB + (qs + 1) * 128],
                            AF.Copy)
                    y_ps = psum_s.tile([128, 1024], f32, tag="s",
                                       name=f"psy{qb}_{qs}")
                    for fb in range(2):
                        for c in range(8):
                            nc.tensor.matmul(
                                y_ps[:, fb * 512:(fb + 1) * 512],
                                ot_qs[:, c, :],
                                wio[:, c, fb * 512:(fb + 1) * 512],
                                start=(c == 0), stop=(c == 7))
                    y_sb = work.tile([128, 1024], f32, tag="y", bufs=2,
                                     name=f"y{qb}_{qs}")
                    nc.vector.tensor_add(y_sb[:], y_ps[:], bob_t[:])
                    nc.sync.dma_start(
                        out_d[q0 + qs * 128: q0 + (qs + 1) * 128, :], y_sb[:])

    nc.compile()
    return nc


def _get_nc():
    if "nc" not in _CACHE:
        _CACHE["nc"] = _build_bass()
    return _CACHE["nc"]


def _make_in_maps(query, key, value, W_split, b_split, W_o, b_o):
    query = np.asarray(query, np.float32)
    key = np.asarray(key, np.float32)
    value = np.asarray(value, np.float32)
    W_split = np.asarray(W_split, np.float32)
    b_split = np.asarray(b_split, np.float32)
    W_o = np.asarray(W_o, np.float32)
    b_o = np.asarray(b_o, np.float32)

    wsT = np.ascontiguousarray(W_split.T)
    woT = np.ascontiguousarray(W_o.T)
    bq = np.ascontiguousarray((b_split / 8.0).reshape(8, 128).T)
    bk = np.ascontiguousarray(b_split.reshape(8, 128).T)
    bvb = np.ascontiguousarray(np.broadcast_to(b_split, (128, D)))
    bob = np.ascontiguousarray(np.broadcast_to(b_o, (128, D)))

    kTs = [np.ascontiguousarray(key[:, b, :].T) for b in range(BATCH)]
    vTs = [np.ascontiguousarray(value[:, b, :].T) for b in range(BATCH)]
    in_maps = []
    for d in range(NCORES):
        b, qc = d // 2, d % 2
        xqT = np.ascontiguousarray(query[qc * QCH:(qc + 1) * QCH, b, :].T)
        in_maps.append({
            "xqT": xqT, "xkT": kTs[b], "xvT": vTs[b],
            "wsT": wsT, "woT": woT,
            "bq": bq, "bk": bk, "bvb": bvb, "bob": bob,
        })
    return in_maps


def kernel_with_results(trace=False, **inputs):
    from concourse.bass_utils import run_bass_kernel_spmd

    nc = _get_nc()
    in_maps = _make_in_maps(**inputs)
    last_exc = None
    for _attempt in range(3):
        try:
            res = run_bass_kernel_spmd(nc, in_maps,
                                       core_ids=list(range(NCORES)),
                                       trace=trace)
            break
        except Exception as exc:  # rare transient device fault -> retry
            last_exc = exc
    else:
        raise last_exc
    out = np.empty((SEQ, BATCH, D), np.float32)
    for d in range(NCORES):
        b, qc = d // 2, d % 2
        out[qc * QCH:(qc + 1) * QCH, b, :] = res.results[d]["out"]
    return out, res


def kernel(**inputs):
    out, _ = kernel_with_results(trace=False, **inputs)
    return out


# revision 15
# speedup vs baseline: 2.7586x; 1.4484x over previous
"""Trainium2 Bass kernel for nn_MultiHeadAttention_87239375716860.

MHA with the reference's quirk: softmax normalizes over the HEADS axis
(score[q,k,b,h], softmax(axis=-1) -> over h), not over keys.

Sharding (no collectives): 8 cores = 4 batches x 2 query-halves.
Core d = 2*b + qc handles batch b, queries [qc*1024, (qc+1)*1024).
Each core projects its batch's full K/V (duplicated between the two
q-half cores) so the softmax-over-heads and the k-contraction are
fully local.

Per-core dataflow (layouts picked so NO on-device transposes are
needed; the host supplies x^T and W^T):
  qT[e,s] = Ws^T-proj of xq^T (scale 1/8 + bias folded in), bf16
  kT[e,s] = Ws^T-proj of xk^T, bf16
  v[s,e]  = proj of xv^T (bias added), bf16
  per (q-block, k-tile, head): s^T[k,q] = kT-chunk @ qT-chunk (psum)
  e = exp(s^T) -> bf16;  Z[k,q] = sum_h e;  a = e * (1/Z)  (bf16)
  o^T[e,q] += v-chunk^T @ a_h   (psum accumulate over k-tiles)
  y[q,f]  = o^T-slices @ Wo^T + b_o -> HBM   (fp32r matmuls)

HW hazard note: matmuls whose SBUF operands or PSUM outputs sit at
partition offset 64 crash the PE when the offset alternates between
consecutive matmuls.  All matmuls here therefore use full-height
(offset-0) operands; per-head selection is done by zero-padding the
unused head-half of q (scores) and v (attn@v) - zeros contribute
nothing to the contraction / accumulation.
"""

import numpy as np

SEQ = 2048
BATCH = 4
D = 1024
H = 16
DK = 64
QCH = 1024          # queries per core
NCORES = 8
QB = 256            # q-block size in phase 2
NQB = QCH // QB     # 4
NKT = SEQ // 128    # 16 k-tiles

_CACHE = {}


def _build_bass():
    """Build + schedule the per-core Bass program (SPMD: same NEFF on all
    8 cores, different input data)."""
    from contextlib import ExitStack

    import concourse.tile as tile
    from concourse import bacc, mybir

    f32 = mybir.dt.float32
    f32r = mybir.dt.float32r
    bf16 = mybir.dt.bfloat16
    AF = mybir.ActivationFunctionType

    nc = bacc.Bacc("TRN2", target_bir_lowering=False, debug=False,
                   num_devices=NCORES)

    xqT_d = nc.dram_tensor("xqT", [D, QCH], f32r, kind="ExternalInput").ap()
    xkT_d = nc.dram_tensor("xkT", [D, SEQ], f32r, kind="ExternalInput").ap()
    xvT_d = nc.dram_tensor("xvT", [D, SEQ], f32r, kind="ExternalInput").ap()
    wsT_d = nc.dram_tensor("wsT", [D, D], f32r, kind="ExternalInput").ap()
    woT_d = nc.dram_tensor("woT", [D, D], f32r, kind="ExternalInput").ap()
    bq_d = nc.dram_tensor("bq", [128, 8], f32, kind="ExternalInput").ap()
    bk_d = nc.dram_tensor("bk", [128, 8], f32, kind="ExternalInput").ap()
    bvb_d = nc.dram_tensor("bvb", [128, D], f32, kind="ExternalInput").ap()
    bob_d = nc.dram_tensor("bob", [128, D], f32, kind="ExternalInput").ap()
    out_d = nc.dram_tensor("out", [QCH, D], f32, kind="ExternalOutput").ap()

    with tile.TileContext(nc) as tc, ExitStack() as ctx:
        # ---------------- pools ----------------
        persist = ctx.enter_context(tc.tile_pool(name="persist", bufs=1))
        psum_s = ctx.enter_context(tc.tile_pool(name="psum_s", space="PSUM", bufs=2))
        psum_o = ctx.enter_context(tc.tile_pool(name="psum_o", space="PSUM", bufs=1))

        # persistent SBUF tensors.  `wio` holds Ws^T during phase 1 and is
        # overwritten with Wo^T once the projections no longer need Ws.
        wio = persist.tile([128, 8, D], f32r, name="wio")
        # qT stored twice with the other head-half zeroed (see module
        # doc), interleaved [p, chunk, parity, s] so a head-pair's scores
        # are one N=512 matmul.
        qT2 = persist.tile([128, 8, 2, QCH], bf16, name="qT2")
        kT = persist.tile([128, 8, SEQ], bf16, name="kT")
        # v per (kt, chunk) stored as [E | Z | O]: even head's lhsT is
        # cols 0:2 ([real | zeros]), odd head's is cols 1:3 ([zeros |
        # real]) -- both contiguous, sharing one zero block.
        vz = persist.tile([128, NKT, 8, 3, DK], bf16, name="vz")
        bq_t = persist.tile([128, 8], f32, name="bq_t")
        bk_t = persist.tile([128, 8], f32, name="bk_t")
        bvb_t = persist.tile([128, 8, 2, DK], f32, name="bvb_t")

        nc.sync.dma_start(wio[:], wsT_d.rearrange("(c p) e -> p c e", p=128))
        nc.gpsimd.memset(qT2[64:128, :, 0, :], 0.0)
        nc.gpsimd.memset(qT2[0:64, :, 1, :], 0.0)
        nc.gpsimd.memset(vz[:, :, :, 1, :], 0.0)
        nc.sync.dma_start(bq_t[:], bq_d)
        nc.sync.dma_start(bk_t[:], bk_d)
        nc.sync.dma_start(bvb_t[:], bvb_d.rearrange("p (c t k) -> p c t k",
                                                    c=8, t=2))

        # ---------------- phase 1: projections ----------------
        with tc.tile_pool(name="stream", bufs=1) as stream:
            # qT[e,s]: lhsT = wsT (d-part, e-cols), rhs = xqT (d-part, s-cols)
            for sb in range(QCH // 512):
                xq_s = stream.tile([128, 8, 512], f32r, tag="xs", bufs=2,
                                   name=f"xq{sb}")
                nc.sync.dma_start(
                    xq_s[:], xqT_d[:, sb * 512:(sb + 1) * 512]
                    .rearrange("(c p) s -> p c s", p=128))
                for c in range(8):
                    ps = psum_s.tile([128, 1024], f32, tag="s",
                                     name=f"psq{sb}_{c}")
                    for dch in range(8):
                        nc.tensor.matmul(
                            ps[:, 0:512],
                            wio[:, dch, c * 128:(c + 1) * 128],
                            xq_s[:, dch, :],
                            start=(dch == 0), stop=(dch == 7))
                    nc.scalar.activation(
                        qT2[0:64, c, 0, sb * 512:(sb + 1) * 512],
                        ps[0:64, 0:512],
                        AF.Identity, bias=bq_t[0:64, c:c + 1], scale=0.125)
                    nc.scalar.activation(
                        qT2[64:128, c, 1, sb * 512:(sb + 1) * 512],
                        ps[64:128, 0:512],
                        AF.Identity, bias=bq_t[64:128, c:c + 1], scale=0.125)

            # kT[e,s]
            for sb in range(SEQ // 512):
                xk_s = stream.tile([128, 8, 512], f32r, tag="xs", bufs=2,
                                   name=f"xk{sb}")
                nc.sync.dma_start(
                    xk_s[:], xkT_d[:, sb * 512:(sb + 1) * 512]
                    .rearrange("(c p) s -> p c s", p=128))
                for c in range(8):
                    ps = psum_s.tile([128, 1024], f32, tag="s",
                                     name=f"psk{sb}_{c}")
                    for dch in range(8):
                        nc.tensor.matmul(
                            ps[:, 0:512],
                            wio[:, dch, c * 128:(c + 1) * 128],
                            xk_s[:, dch, :],
                            start=(dch == 0), stop=(dch == 7))
                    nc.scalar.activation(
                        kT[:, c, sb * 512:(sb + 1) * 512], ps[:, 0:512],
                        AF.Identity, bias=bk_t[:, c:c + 1])

            # v[s,e] bf16 (x2, complementary head-halves zeroed)
            for kch in range(NKT):
                xv_s = stream.tile([128, 8, 128], f32r, tag="xv", bufs=2,
                                   name=f"xv{kch}")
                nc.sync.dma_start(
                    xv_s[:], xvT_d[:, kch * 128:(kch + 1) * 128]
                    .rearrange("(c p) k -> p c k", p=128))
                ps = psum_s.tile([128, 1024], f32, tag="s", name=f"psv{kch}")
                for eb in range(2):
                    for dch in range(8):
                        nc.tensor.matmul(
                            ps[:, eb * 512:(eb + 1) * 512],
                            xv_s[:, dch, :],
                            wio[:, dch, eb * 512:(eb + 1) * 512],
                            start=(dch == 0), stop=(dch == 7))
                pv = ps.rearrange("p (c t k) -> p c t k", c=8, t=2)
                nc.vector.tensor_add(vz[:, kch, :, 0, :], pv[:, :, 0, :],
                                     bvb_t[:, :, 0, :])
                nc.vector.tensor_add(vz[:, kch, :, 2, :], pv[:, :, 1, :],
                                     bvb_t[:, :, 1, :])

        # Ws no longer needed -- load Wo^T into the same tile.
        nc.sync.dma_start(wio[:], woT_d.rearrange("(c p) e -> p c e", p=128))

        # ---------------- phase 2+3: attention + out-proj ----------------
        with tc.tile_pool(name="work", bufs=1) as work:
            bob_t = work.tile([128, D], f32, tag="bob", bufs=1, name="bob_t")
            nc.sync.dma_start(bob_t[:], bob_d)
            for qb in range(NQB):
                q0 = qb * QB
                oT_ps = psum_o.tile([128, 8 * QB], f32, tag="ot",
                                    name=f"ot{qb}")
                for kt in range(NKT):
                    e = work.tile([128, H, QB], bf16, tag="e", bufs=3,
                                  name=f"e{qb}_{kt}")
                    # scores (4 heads = 2 chunk-pairs per psum tile) + exp.
                    # One N=512 matmul per chunk covers its even+odd head
                    # (qT2 parity axis); the unused head-half of q is zero.
                    for hg in range(4):
                        ps = psum_s.tile([128, 1024], f32, tag="s",
                                         name=f"pss{qb}_{kt}_{hg}")
                        for cl in range(2):
                            c = hg * 2 + cl
                            nc.tensor.matmul(
                                ps[:, cl * 512:(cl + 1) * 512],
                                kT[:, c, kt * 128:(kt + 1) * 128],
                                qT2[:, c, :, q0:q0 + QB],
                                start=True, stop=True)
                        nc.scalar.activation(
                            e[:, hg * 4:(hg + 1) * 4, :], ps[:, :], AF.Exp)
                    # Z = sum over heads (tree); R = 1/Z
                    t1 = work.tile([128, 2, 4, QB], bf16, tag="t1", bufs=2,
                                   name=f"t1_{qb}_{kt}")
                    nc.vector.tensor_add(t1[:, 0, :, :], e[:, 0:4, :],
                                         e[:, 4:8, :])
                    nc.vector.tensor_add(t1[:, 1, :, :], e[:, 8:12, :],
                                         e[:, 12:16, :])
                    t2 = work.tile([128, 4, QB], bf16, tag="t2", bufs=1,
                                   name=f"t2_{qb}_{kt}")
                    nc.vector.tensor_add(t2[:], t1[:, 0, :, :], t1[:, 1, :, :])
                    t3 = work.tile([128, 2, QB], bf16, tag="t3", bufs=1,
                                   name=f"t3_{qb}_{kt}")
                    nc.gpsimd.tensor_add(t3[:], t2[:, 0:2, :], t2[:, 2:4, :])
                    zf = work.tile([128, QB], f32, tag="zf", bufs=2,
                                   name=f"zf{qb}_{kt}")
                    nc.vector.tensor_add(zf[:], t3[:, 0, :], t3[:, 1, :])
                    rf = work.tile([128, QB], f32, tag="rf", bufs=2,
                                   name=f"rf{qb}_{kt}")
                    nc.vector.reciprocal_approx_fast(rf[:], zf[:])
                    rb = work.tile([128, QB], bf16, tag="rb", bufs=1,
                                   name=f"rb{qb}_{kt}")
                    nc.vector.tensor_copy(rb[:], rf[:])
                    # a = e * R  (in place), R broadcast over heads;
                    # split so attn@v on the first half starts earlier
                    nc.vector.tensor_mul(
                        e[:, 0:8, :], e[:, 0:8, :],
                        rb[:].unsqueeze(1).broadcast_to([128, 8, QB]))
                    nc.vector.tensor_mul(
                        e[:, 8:16, :], e[:, 8:16, :],
                        rb[:].unsqueeze(1).broadcast_to([128, 8, QB]))
                    # o^T[e,q] accumulation, full height; the unused
                    # head-half of v is zero so it adds nothing.
                    # PSUM bank = h//4; start/stop on first/last writer.
                    for h in range(H):
                        c, t0 = h // 2, h % 2
                        nc.tensor.matmul(
                            oT_ps[:, c * QB:(c + 1) * QB],
                            vz[:, kt, c, t0:t0 + 2, :],
                            e[:, h, :],
                            start=(kt == 0 and h % 4 == 0),
                            stop=(kt == NKT - 1 and h % 4 == 3),
                            skip_group_check=True)
                # phase 3: evacuate o^T, out-projection, bias, store
                for qs in range(QB // 128):
                    ot_qs = work.tile([128, 8, 128], f32r, tag="ot_sb",
                                      bufs=1, name=f"otsb{qb}_{qs}")
                    for c in range(8):
                        nc.scalar.activation(
                            ot_qs[:, c, :],
                            oT_ps[:, c * QB + qs * 128:
                                  c * QB + (qs + 1) * 128],
                            AF.Copy)
                    y_ps = psum_s.tile([128, 1024], f32, tag="s",
                                       name=f"psy{qb}_{qs}")
                    for fb in range(2):
                        for c in range(8):
                            nc.tensor.matmul(
                                y_ps[:, fb * 512:(fb + 1) * 512],
                                ot_qs[:, c, :],
                                wio[:, c, fb * 512:(fb + 1) * 512],
                                start=(c == 0), stop=(c == 7))
                    y_sb = work.tile([128, 1024], f32, tag="y", bufs=2,
                                     name=f"y{qb}_{qs}")
                    nc.vector.tensor_add(y_sb[:], y_ps[:], bob_t[:])
                    nc.sync.dma_start(
                        out_d[q0 + qs * 128: q0 + (qs + 1) * 128, :], y_sb[:])

    nc.compile()
    return nc


def _get_nc():
    if "nc" not in _CACHE:
        _CACHE["nc"] = _build_bass()
    return _CACHE["nc"]


def _make_in_maps(query, key, value, W_split, b_split, W_o, b_o):
    query = np.asarray(query, np.float32)
    key = np.asarray(key, np.float32)
    value = np.asarray(value, np.float32)
    W_split = np.asarray(W_split, np.float32)
    b_split = np.asarray(b_split, np.float32)
    W_o = np.asarray(W_o, np.float32)
    b_o = np.asarray(b_o, np.float32)

    wsT = np.ascontiguousarray(W_split.T)
    woT = np.ascontiguousarray(W_o.T)
    bq = np.ascontiguousarray((b_split / 8.0).reshape(8, 128).T)
    bk = np.ascontiguousarray(b_split.reshape(8, 128).T)
    bvb = np.ascontiguousarray(np.broadcast_to(b_split, (128, D)))
    bob = np.ascontiguousarray(np.broadcast_to(b_o, (128, D)))

    kTs = [np.ascontiguousarray(key[:, b, :].T) for b in range(BATCH)]
    vTs = [np.ascontiguousarray(value[:, b, :].T) for b in range(BATCH)]
    in_maps = []
    for d in range(NCORES):
        b, qc = d // 2, d % 2
        xqT = np.ascontiguousarray(query[qc * QCH:(qc + 1) * QCH, b, :].T)
        in_maps.append({
            "xqT": xqT, "xkT": kTs[b], "xvT": vTs[b],
            "wsT": wsT, "woT": woT,
            "bq": bq, "bk": bk, "bvb": bvb, "bob": bob,
        })
    return in_maps


def kernel_with_results(trace=False, **inputs):
    from concourse.bass_utils import run_bass_kernel_spmd

    nc = _get_nc()
    in_maps = _make_in_maps(**inputs)
    last_exc = None
    for _attempt in range(3):
        try:
            res = run_bass_kernel_spmd(nc, in_maps,
                                       core_ids=list(range(NCORES)),
                                       trace=trace)
            break
        except Exception as exc:  # rare transient device fault -> retry
            last_exc = exc
    else:
        raise last_exc
    out = np.empty((SEQ, BATCH, D), np.float32)
    for d in range(NCORES):
        b, qc = d // 2, d % 2
        out[qc * QCH:(qc + 1) * QCH, b, :] = res.results[d]["out"]
    return out, res


def kernel(**inputs):
    out, _ = kernel_with_results(trace=False, **inputs)
    return out
